# revision 9
# baseline (speedup 1.0000x reference)
"""Trainium2 Bass kernel for BinaryMemoryRNN (scatter_memory).

Math (per batch row b):
    logits = h_prev @ M_w.T + M_b                 [B, 10]
    bits   = (sigmoid(logits) > 0.5) = (logits > -M_b)
    index  = sum(bits * 2^(9-i))                  [B] in [0, 1023]
    h_mem  = memory[index]
    pre    = x @ W_w.T + W_b + h_prev @ U_w.T + U_b + h_mem @ Q_w.T + Q_b
    out    = sigmoid(LayerNorm(pre) * ln_g + ln_b)

Key transforms:
  - R = memory @ Q_w.T + combined_bias precomputed once per core; the
    gather+matmul becomes a row gather of R feeding an add.
  - All three big GEMMs run in fp8 e4m3 with perf_mode=DoubleRow
    (K=256 per matmul, ~1.5x bf16 throughput). Weights are scaled by 64
    into fp8 normal range; the 2^-6 descale is folded into the epilogue
    scalar_tensor_tensor ops. End-to-end rel err ~1.5e-2 (budget 2e-2).
  - Address logits run transposed (M_w stationary [128,10], h streaming
    512 batch cols) in fp32r: one pass at full rate, and the tiny
    truncation error (fp22) flips ~1 row in 16k. Indices come back to
    partition-major via a 2-DMA DRAM bounce.
  - Epilogue: pre in bf16, bn_stats/aggr, ACT Rsqrt for 1/std, single
    fused sigmoid activation, y written as bf16.

Sharding: data-parallel over batch across 8 cores (2048 rows each);
weights + memory table replicated.
"""

import os as _os
import numpy as np
import ml_dtypes
from contextlib import ExitStack

import concourse.bass as bass
import concourse.mybir as mybir
import concourse.tile as tile
from concourse import bacc
from concourse import bass_utils

P = 128            # partitions
NCORES = 8
B = 16384          # full batch
BC = B // NCORES   # batch rows per core (2048)
BT = BC // P       # b-tiles per core (16)
KC4 = 4            # contraction chunks of 256 (DoubleRow)
KC8 = 8            # contraction chunks of 128 (logits)
H = 1024
NB = 10            # address bits
MEM = 1024         # memory rows
LN_EPS = 1e-5
GROUP = 4          # b-tiles per logits group
WSCALE = 64.0      # fp8 weight scale (power of 2)
INV_WSCALE = 1.0 / WSCALE

F32 = mybir.dt.float32
F32R = mybir.dt.float32r
BF16 = mybir.dt.bfloat16
F8 = mybir.dt.float8e4
I32 = mybir.dt.int32
F8NP = ml_dtypes.float8_e4m3fn
BF16_NP = ml_dtypes.bfloat16
DR = mybir.MatmulPerfMode.DoubleRow

_CACHE = {}


def _bcast_ap(handle, n):
    """[n] DRAM tensor -> [P, n] AP broadcast across partitions (step 0)."""
    h = handle.ap()
    return bass.AP(tensor=h.tensor, offset=h.offset, ap=[[0, P], *list(h.ap)])


def build_nc(zero_affine=True, r_bf16=False, zero_cb=True):
    nc = bacc.Bacc("TRN2", debug=False, enable_asserts=False)

    xT = nc.dram_tensor("xT", [P, BT, KC4, 2, P], F8, kind="ExternalInput")
    hT = nc.dram_tensor("hT", [P, BT, KC4, 2, P], F8, kind="ExternalInput")
    hTf = nc.dram_tensor("hTf", [P, BT, KC8, P], F32R, kind="ExternalInput")
    if r_bf16:
        Qt = nc.dram_tensor("Qt", [P, KC8, H], BF16, kind="ExternalInput")
        Mem = nc.dram_tensor("Mem", [P, KC8, MEM], BF16, kind="ExternalInput")
    else:
        MQ = nc.dram_tensor("MQ", [P, KC4, 2, MEM + H], F8, kind="ExternalInput")
    WU = nc.dram_tensor("WU", [P, KC4, 2, 2 * H], F8, kind="ExternalInput")
    Mt = nc.dram_tensor("Mt", [P, KC8, NB], F32R, kind="ExternalInput")
    if not zero_cb:
        cbP = nc.dram_tensor("cbP", [P, H], F32, kind="ExternalInput")
    if not zero_affine:
        lngP = nc.dram_tensor("lngP", [P, H], F32, kind="ExternalInput")
        lnbP = nc.dram_tensor("lnbP", [P, H], F32, kind="ExternalInput")
    negmb = nc.dram_tensor("negmb", [NB], F32, kind="ExternalInput")
    powers = nc.dram_tensor("powers", [NB], F32, kind="ExternalInput")
    y = nc.dram_tensor("y", [BC, H], BF16, kind="ExternalOutput")
    R = nc.dram_tensor("Rtab", [MEM, H], BF16, kind="Internal")
    scr = nc.dram_tensor("scr", [BT * P], I32, kind="Internal")
    dbg = bool(int(_os.environ.get("K_DEBUG", "0")))
    if dbg:
        idx_out = nc.dram_tensor("idx_out", [BT * P], I32, kind="ExternalOutput")
        R_out = nc.dram_tensor("R_out", [MEM, H], BF16, kind="ExternalOutput")
    wsink_d = nc.dram_tensor("wsink", [P, 1], F32, kind="Internal")
    y_ap = y.ap()
    R_ap = R.ap()
    scr_ap = scr.ap()

    rscale = INV_WSCALE  # psum holds 64x the true product

    with tile.TileContext(nc) as tc, ExitStack() as ctx:
        wpool = ctx.enter_context(tc.tile_pool(name="weights", bufs=1))
        work = ctx.enter_context(tc.tile_pool(name="work", bufs=3))
        rgp = ctx.enter_context(tc.tile_pool(name="rgp", bufs=4))
        hpool = ctx.enter_context(tc.tile_pool(name="hpool", bufs=2))
        ipool = ctx.enter_context(tc.tile_pool(name="ipool", bufs=2))
        epil = ctx.enter_context(tc.tile_pool(name="epil", bufs=3))
        small = ctx.enter_context(tc.tile_pool(name="small", bufs=8))
        psum = ctx.enter_context(tc.tile_pool(name="psum", bufs=2, space="PSUM"))

        # ---- warmup: dummy matmuls with no DMA deps keep the PE busy and
        # trip the HAM clock gate while the first weight chunks stream in.
        wu_l = wpool.tile([P, 2, P], F8)
        wu_r = wpool.tile([P, 2, 512], F8)
        nc.gpsimd.memset(wu_l[:], 0)
        nc.gpsimd.memset(wu_r[:], 0)
        n_wu = int(_os.environ.get("K_WU", "8"))
        ps_w = psum.tile([P, 512], F32, tag="psL", space="PSUM")
        for _ in range(n_wu):
            nc.tensor.matmul(out=ps_w[:], lhsT=wu_l[:], rhs=wu_r[:],
                             start=True, stop=True, perf_mode=DR)
        wsink = wpool.tile([P, 1], F32)
        nc.vector.tensor_copy(out=wsink[:], in_=ps_w[:, 0:1])
        nc.scalar.dma_start(out=wsink_d.ap()[:, :], in_=wsink[:])

        # ---- resident constants; mem/q first so the R phase starts early
        if r_bf16:
            mem_sb = wpool.tile([P, KC8, MEM], BF16)
            q_sb = wpool.tile([P, KC8, H], BF16)
            for kc in range(KC8):
                nc.sync.dma_start(out=mem_sb[:, kc, :], in_=Mem.ap()[:, kc, :])
                nc.sync.dma_start(out=q_sb[:, kc, :], in_=Qt.ap()[:, kc, :])
        else:
            # single DMA: one completion sem gates every R-phase LDW and MM
            # together (a stalled DoubleRow matmul with runnable successor
            # LDWEIGHTS clobbers the PE weight buffer).
            mq_sb = wpool.tile([P, KC4, 2, MEM + H], F8)
            for kc in range(KC4):
                nc.sync.dma_start(out=mq_sb[:, kc, :, :],
                                  in_=MQ.ap()[:, kc, :, :])
            mem_sb = mq_sb
            q_sb = mq_sb
        wu_sb = wpool.tile([P, KC4, 2, 2 * H], F8)
        for kc in range(KC4):
            nc.sync.dma_start(out=wu_sb[:, kc, :, :], in_=WU.ap()[:, kc, :, :])
        w_sb = wu_sb
        u_sb = wu_sb
        mw_sb = wpool.tile([P, KC8, NB], F32R)
        nc.sync.dma_start(out=mw_sb[:], in_=Mt.ap()[:, :, :])

        if not zero_cb:
            cbb = wpool.tile([P, H], F32)
            nc.sync.dma_start(out=cbb[:], in_=cbP.ap())
        if not zero_affine:
            gb = wpool.tile([P, H], F32)
            bb = wpool.tile([P, H], F32)
            nc.sync.dma_start(out=gb[:], in_=lngP.ap())
            nc.sync.dma_start(out=bb[:], in_=lnbP.ap())
        eps = wpool.tile([P, 1], F32)
        nc.vector.memset(eps[:], LN_EPS)
        nmb_c = wpool.tile([NB, 1], F32)
        pw_c = wpool.tile([NB, 1], F32)
        nc.sync.dma_start(out=nmb_c[:], in_=negmb.ap()[:, None])
        nc.sync.dma_start(out=pw_c[:], in_=powers.ap()[:, None])
        ones10 = wpool.tile([NB, 1], F32)
        nc.vector.memset(ones10[:], 1.0)

        # ---- phase 1: R = (memory @ Q_w.T) * 2^-6 + combined_bias ----
        # mt-outer, halves alternate psA/psB so the drain of one unit
        # overlaps the matmuls of the next.
        for mt in range(KC8):
            for half in range(2):
                hs = slice(half * 512, (half + 1) * 512)
                tag = "psA" if half == 0 else "psB"
                ps_r = psum.tile([P, 512], F32, tag=tag, space="PSUM", bufs=3)
                if r_bf16:
                    for kc in range(KC8):
                        nc.tensor.matmul(out=ps_r[:],
                                         lhsT=mem_sb[:, kc, mt * P:(mt + 1) * P],
                                         rhs=q_sb[:, kc, hs],
                                         start=(kc == 0), stop=(kc == KC8 - 1))
                else:
                    qs = slice(MEM + half * 512, MEM + (half + 1) * 512)
                    for kc in range(KC4):
                        nc.tensor.matmul(out=ps_r[:],
                                         lhsT=mem_sb[:, kc, :, mt * P:(mt + 1) * P],
                                         rhs=q_sb[:, kc, :, qs],
                                         start=(kc == 0), stop=(kc == KC4 - 1),
                                         perf_mode=DR)
                r_sb = work.tile([P, 512], BF16, tag="rtile")
                if zero_cb:
                    nc.vector.tensor_scalar(
                        out=r_sb[:], in0=ps_r[:], scalar1=rscale, scalar2=None,
                        op0=mybir.AluOpType.mult)
                else:
                    nc.vector.scalar_tensor_tensor(
                        out=r_sb[:], in0=ps_r[:], scalar=rscale, in1=cbb[:, hs],
                        op0=mybir.AluOpType.mult, op1=mybir.AluOpType.add)
                nc.scalar.dma_start(out=R_ap[mt * P:(mt + 1) * P, hs], in_=r_sb[:])
                if dbg:
                    nc.scalar.dma_start(out=R_out.ap()[mt * P:(mt + 1) * P, hs],
                                        in_=r_sb[:])

        # ---- phase 2: 4 groups of 4 b-tiles ----
        def tile_mms(bt):
            """DMA + the 16 DoubleRow matmuls for one b-tile; returns psums."""
            xb = work.tile([P, KC4, 2, P], F8, tag="xb")
            hb = work.tile([P, KC4, 2, P], F8, tag="hb")
            nc.sync.dma_start(out=xb[:], in_=xT.ap()[:, bt, :, :, :])
            nc.sync.dma_start(out=hb[:], in_=hT.ap()[:, bt, :, :, :])
            ps0 = psum.tile([P, 512], F32, tag="psA", space="PSUM", bufs=3)
            ps1 = psum.tile([P, 512], F32, tag="psB", space="PSUM", bufs=3)
            for kc in range(KC4):
                nc.tensor.matmul(out=ps0[:], lhsT=xb[:, kc, :, :],
                                 rhs=w_sb[:, kc, :, 0:512],
                                 start=(kc == 0), stop=False, perf_mode=DR)
                nc.tensor.matmul(out=ps1[:], lhsT=xb[:, kc, :, :],
                                 rhs=w_sb[:, kc, :, 512:1024],
                                 start=(kc == 0), stop=False, perf_mode=DR)
            for kc in range(KC4):
                nc.tensor.matmul(out=ps0[:], lhsT=hb[:, kc, :, :],
                                 rhs=u_sb[:, kc, :, H:H + 512],
                                 start=False, stop=(kc == KC4 - 1), perf_mode=DR)
                nc.tensor.matmul(out=ps1[:], lhsT=hb[:, kc, :, :],
                                 rhs=u_sb[:, kc, :, H + 512:2 * H],
                                 start=False, stop=(kc == KC4 - 1), perf_mode=DR)
            return ps0, ps1

        def tile_epilogue(bt, ps0, ps1, rg):
            pre = epil.tile([P, H], BF16, tag="pre")
            nc.vector.scalar_tensor_tensor(
                out=pre[:, 0:512], in0=ps0[:], scalar=rscale, in1=rg[:, 0:512],
                op0=mybir.AluOpType.mult, op1=mybir.AluOpType.add)
            nc.vector.scalar_tensor_tensor(
                out=pre[:, 512:1024], in0=ps1[:], scalar=rscale,
                in1=rg[:, 512:1024],
                op0=mybir.AluOpType.mult, op1=mybir.AluOpType.add)

            stats = small.tile([P, 2, 6], F32, tag="stats")
            mv = small.tile([P, 2], F32, tag="mv")
            nc.vector.bn_stats(out=stats[:, 0, :], in_=pre[:, 0:512])
            nc.vector.bn_stats(out=stats[:, 1, :], in_=pre[:, 512:1024])
            nc.vector.bn_aggr(out=mv[:], in_=stats[:])

            sd = small.tile([P, 1], F32, tag="sd")
            rstd = small.tile([P, 1], F32, tag="rstd")
            nc.scalar.activation(out=sd[:], in_=mv[:, 1:2],
                                 func=mybir.ActivationFunctionType.Sqrt,
                                 bias=eps[:], scale=1.0)
            nc.vector.reciprocal(out=rstd[:], in_=sd[:])
            if zero_affine:
                nmr = small.tile([P, 1], F32, tag="nmr")
                nc.vector.scalar_tensor_tensor(
                    out=nmr[:], in0=mv[:, 0:1], scalar=-1.0, in1=rstd[:],
                    op0=mybir.AluOpType.mult, op1=mybir.AluOpType.mult)
                ob = work.tile([P, H], BF16, tag="ob")
                nc.scalar.activation(out=ob[:], in_=pre[:],
                                     func=mybir.ActivationFunctionType.Sigmoid,
                                     bias=nmr[:], scale=rstd[:])
                nc.scalar.dma_start(out=y_ap[bt * P:(bt + 1) * P, :], in_=ob[:])
            else:
                t = epil.tile([P, H], F32, tag="tgen")
                nc.vector.scalar_tensor_tensor(
                    out=t[:], in0=pre[:], scalar=mv[:, 0:1], in1=gb[:],
                    op0=mybir.AluOpType.subtract, op1=mybir.AluOpType.mult)
                nc.vector.scalar_tensor_tensor(
                    out=t[:], in0=t[:], scalar=rstd[:], in1=bb[:],
                    op0=mybir.AluOpType.mult, op1=mybir.AluOpType.add)
                ob = work.tile([P, H], BF16, tag="ob")
                nc.scalar.activation(out=ob[:], in_=t[:],
                                     func=mybir.ActivationFunctionType.Sigmoid)
                nc.scalar.dma_start(out=y_ap[bt * P:(bt + 1) * P, :], in_=ob[:])

        for g0 in range(0, BT, GROUP):
            n = GROUP * P  # 512 batch cols per group
            # transposed fp32r logits: M_w stationary, h streaming
            hfg = hpool.tile([P, KC8, GROUP, P], F32R, tag="hfg")
            for kc in range(KC8):
                nc.sync.dma_start(out=hfg[:, kc, :, :],
                                  in_=hTf.ap()[:, g0:g0 + GROUP, kc, :])
            psLT = psum.tile([NB, 512], F32, tag="psL", space="PSUM")
            for kc in range(KC8):
                nc.tensor.matmul(out=psLT[:, 0:n], lhsT=mw_sb[:, kc, :],
                                 rhs=hfg[:, kc, :, :], start=(kc == 0),
                                 stop=(kc == KC8 - 1))

            # first tile's matmuls cover the latency of the index pipeline
            ps0_0, ps1_0 = tile_mms(g0 + 0)

            # bits -> index (fp32, exact for integers)
            bitsT = ipool.tile([NB, 512], F32, tag="bitsT")
            nc.vector.tensor_scalar(out=bitsT[:, 0:n], in0=psLT[:, 0:n],
                                    scalar1=nmb_c[:], scalar2=pw_c[:],
                                    op0=mybir.AluOpType.is_gt,
                                    op1=mybir.AluOpType.mult)
            psI = psum.tile([1, 512], F32, tag="psL", space="PSUM")
            nc.tensor.matmul(out=psI[0:1, 0:n], lhsT=ones10[:],
                             rhs=bitsT[:, 0:n], start=True, stop=True)
            idxf = ipool.tile([1, 512], I32, tag="idxf")
            nc.vector.tensor_copy(out=idxf[0:1, 0:n], in_=psI[0:1, 0:n])
            # DRAM bounce: [1, 512] free-axis -> [128, 4] partition-major
            nc.scalar.dma_start(out=scr_ap[None, g0 * P:(g0 + GROUP) * P],
                                in_=idxf[0:1, 0:n])
            idxT = small.tile([P, GROUP], I32, tag="idxT")
            tr_ap = bass.AP(tensor=scr_ap.tensor, offset=g0 * P,
                            ap=[[1, P], [P, GROUP]])
            nc.scalar.dma_start(out=idxT[:, :], in_=tr_ap)
            if dbg:
                nc.scalar.dma_start(
                    out=idx_out.ap()[None, g0 * P:(g0 + GROUP) * P],
                    in_=idxf[0:1, 0:n])

            # gathers for all 4 tiles
            rgs = []
            for tb in range(GROUP):
                rg = rgp.tile([P, H], BF16, tag="rg")
                nc.gpsimd.indirect_dma_start(
                    out=rg[:], out_offset=None, in_=R_ap[:, :],
                    in_offset=bass.IndirectOffsetOnAxis(
                        ap=idxT[:, tb:tb + 1], axis=0))
                rgs.append(rg)

            tile_epilogue(g0 + 0, ps0_0, ps1_0, rgs[0])
            for tb in range(1, GROUP):
                ps0, ps1 = tile_mms(g0 + tb)
                tile_epilogue(g0 + tb, ps0, ps1, rgs[tb])

    nc.compile()
    return nc


import os as _os

FLAGS = {
    "r_bf16": bool(int(_os.environ.get("K_RBF16", "0"))),
}


def _get_nc(zero_affine=True, zero_cb=True):
    key = ("nc", zero_affine, zero_cb, tuple(sorted(FLAGS.items())))
    if key not in _CACHE:
        _CACHE[key] = build_nc(zero_affine, zero_cb=zero_cb, **FLAGS)
    return _CACHE[key]


def _pack_act8(a):
    """[BC, 1024] -> [p, bt, kc, ko, bp] fp8, k = kc*256 + ko*128 + p."""
    t = a.reshape(BT, P, KC4, 2, P).transpose(4, 0, 2, 3, 1)
    return np.ascontiguousarray(t).astype(F8NP)


def _pack_w8(w, scale=WSCALE):
    """[n, 1024] -> [p, kc, ko, n] fp8 scaled."""
    t = w.T.reshape(KC4, 2, P, -1).transpose(2, 0, 1, 3)
    return np.clip(np.ascontiguousarray(t) * np.float32(scale),
                   -240, 240).astype(F8NP)


def _pack_wb(w):
    """[n, 1024] -> [p, kc8, n] bf16 (for r_bf16 fallback)."""
    t = w.T.reshape(KC8, P, -1).transpose(1, 0, 2)
    return np.ascontiguousarray(t).astype(BF16_NP)


def _pack_f32(a):
    """[BC, 1024] -> [p, bt, kc8, bp] f32."""
    return np.ascontiguousarray(a.reshape(BT, P, KC8, P).transpose(3, 0, 2, 1))


def prepare_in_maps(inputs):
    x = np.asarray(inputs["x"], np.float32)
    h = np.asarray(inputs["h_prev"], np.float32)
    memory = np.asarray(inputs["memory"], np.float32)
    W_w = np.asarray(inputs["W_w"], np.float32)
    U_w = np.asarray(inputs["U_w"], np.float32)
    Q_w = np.asarray(inputs["Q_w"], np.float32)
    M_w = np.asarray(inputs["M_w"], np.float32)
    W_b = np.asarray(inputs["W_b"], np.float32)
    U_b = np.asarray(inputs["U_b"], np.float32)
    Q_b = np.asarray(inputs["Q_b"], np.float32)
    M_b = np.asarray(inputs["M_b"], np.float32)
    ln_g = np.asarray(inputs["ln_g"], np.float32)
    ln_b = np.asarray(inputs["ln_b"], np.float32)

    cb = (W_b + U_b + Q_b).astype(np.float32)
    zero_cb = bool(np.all(cb == 0.0))
    shared = {
        "WU": np.concatenate([_pack_w8(W_w), _pack_w8(U_w)], axis=3),
        # Mt: [p, kc8, j] fp32 (fp32r bits)
        "Mt": np.ascontiguousarray(
            M_w.T.reshape(KC8, P, NB).transpose(1, 0, 2)),
        "negmb": np.ascontiguousarray(-M_b),
        "powers": (2.0 ** np.arange(NB - 1, -1, -1)).astype(np.float32),
    }
    if FLAGS["r_bf16"]:
        shared["Qt"] = _pack_wb(Q_w)
        shared["Mem"] = _pack_wb(memory)
    else:
        shared["MQ"] = np.concatenate(
            [_pack_w8(memory, scale=1.0), _pack_w8(Q_w)], axis=3)
    if not zero_cb:
        shared["cbP"] = np.ascontiguousarray(np.broadcast_to(cb, (P, H)))
    zero_affine = bool(np.all(ln_g == 1.0) and np.all(ln_b == 0.0))
    if not zero_affine:
        shared["lngP"] = np.ascontiguousarray(np.broadcast_to(ln_g, (P, H)))
        shared["lnbP"] = np.ascontiguousarray(np.broadcast_to(ln_b, (P, H)))
    in_maps = []
    for i in range(NCORES):
        sl = slice(i * BC, (i + 1) * BC)
        m = dict(shared)
        m["xT"] = _pack_act8(x[sl])
        m["hT"] = _pack_act8(h[sl])
        m["hTf"] = _pack_f32(h[sl])
        in_maps.append(m)
    return in_maps


def run(inputs, trace=False, trace_cores=None):
    zero_affine = bool(
        np.all(np.asarray(inputs["ln_g"], np.float32) == 1.0)
        and np.all(np.asarray(inputs["ln_b"], np.float32) == 0.0))
    zero_cb = bool(
        np.all(np.asarray(inputs["W_b"], np.float32) == 0.0)
        and np.all(np.asarray(inputs["U_b"], np.float32) == 0.0)
        and np.all(np.asarray(inputs["Q_b"], np.float32) == 0.0))
    nc = _get_nc(zero_affine, zero_cb)
    in_maps = prepare_in_maps(inputs)
    res = bass_utils.run_bass_kernel_spmd(
        nc, in_maps, core_ids=list(range(NCORES)), trace=trace,
        trace_cores=trace_cores)
    out = np.concatenate(
        [np.asarray(r["y"], dtype=np.float32) for r in res.results], axis=0)
    return out, res


def kernel(**inputs):
    out, _ = run(inputs)
    return out.astype(np.float32)


def enable_profiling():
    """Inject the missing antenv.axon_hooks shim so trace=True works, and
    neutralize the S3 artifact upload (zero-egress container)."""
    import sys
    import types
    try:
        import antenv.axon_hooks  # noqa: F401
    except ImportError:
        mod = types.ModuleType("antenv.axon_hooks")
        _hook = [None]
        mod.set_axon_ntff_profile_hook = lambda h: _hook.__setitem__(0, h)
        mod.get_axon_ntff_profile_hook = lambda: _hook[0]
        sys.modules["antenv.axon_hooks"] = mod
        from trn_agent_boot.trn_boot import _ntff_profile_via_ctypes
        mod.set_axon_ntff_profile_hook(
            _ntff_profile_via_ctypes("/opt/axon/libaxon_pjrt.so"))
    bass_utils.upload_artifacts = lambda d: "local://" + str(d)


# revision 11
# speedup vs baseline: 1.1033x; 1.1033x over previous
"""Trainium2 Bass kernel for BinaryMemoryRNN (scatter_memory).

Math (per batch row b):
    logits = h_prev @ M_w.T + M_b                 [B, 10]
    bits   = (sigmoid(logits) > 0.5) = (logits > -M_b)
    index  = sum(bits * 2^(9-i))                  [B] in [0, 1023]
    h_mem  = memory[index]
    pre    = x @ W_w.T + W_b + h_prev @ U_w.T + U_b + h_mem @ Q_w.T + Q_b
    out    = sigmoid(LayerNorm(pre) * ln_g + ln_b)

Key transforms:
  - R = memory @ Q_w.T + combined_bias precomputed once per core; the
    gather+matmul becomes a row gather of R feeding an add.
  - All three big GEMMs run in fp8 e4m3 with perf_mode=DoubleRow
    (K=256 per matmul, ~1.5x bf16 throughput). Weights are scaled by 64
    into fp8 normal range; the 2^-6 descale is folded into the epilogue
    scalar_tensor_tensor ops. End-to-end rel err ~1.5e-2 (budget 2e-2).
  - Address logits run transposed (M_w stationary [128,10], h streaming
    512 batch cols) in fp32r: one pass at full rate, and the tiny
    truncation error (fp22) flips ~1 row in 16k. Indices come back to
    partition-major via a 2-DMA DRAM bounce.
  - Epilogue: pre in bf16, bn_stats/aggr, ACT Rsqrt for 1/std, single
    fused sigmoid activation, y written as bf16.

Sharding: data-parallel over batch across 8 cores (2048 rows each);
weights + memory table replicated.
"""

import os as _os
import numpy as np
import ml_dtypes
from contextlib import ExitStack

import concourse.bass as bass
import concourse.mybir as mybir
import concourse.tile as tile
from concourse import bacc
from concourse import bass_utils

P = 128            # partitions
NCORES = 8
B = 16384          # full batch
BC = B // NCORES   # batch rows per core (2048)
BT = BC // P       # b-tiles per core (16)
KC4 = 4            # contraction chunks of 256 (DoubleRow)
KC8 = 8            # contraction chunks of 128 (logits)
H = 1024
NB = 10            # address bits
MEM = 1024         # memory rows
LN_EPS = 1e-5
GROUP = 4          # b-tiles per logits group
WSCALE = 64.0      # fp8 weight scale (power of 2)
INV_WSCALE = 1.0 / WSCALE

F32 = mybir.dt.float32
F32R = mybir.dt.float32r
BF16 = mybir.dt.bfloat16
F8 = mybir.dt.float8e4
I32 = mybir.dt.int32
F8NP = ml_dtypes.float8_e4m3fn
BF16_NP = ml_dtypes.bfloat16
DR = mybir.MatmulPerfMode.DoubleRow

_CACHE = {}


def _bcast_ap(handle, n):
    """[n] DRAM tensor -> [P, n] AP broadcast across partitions (step 0)."""
    h = handle.ap()
    return bass.AP(tensor=h.tensor, offset=h.offset, ap=[[0, P], *list(h.ap)])


def build_nc(zero_affine=True, r_bf16=False, zero_cb=True):
    nc = bacc.Bacc("TRN2", debug=False, enable_asserts=False)

    xT = nc.dram_tensor("xT", [P, BT, KC4, 2, P], F8, kind="ExternalInput")
    hT = nc.dram_tensor("hT", [P, BT, KC4, 2, P], F8, kind="ExternalInput")
    hTf = nc.dram_tensor("hTf", [P, BT, KC8, P], F32R, kind="ExternalInput")
    if r_bf16:
        Qt = nc.dram_tensor("Qt", [P, KC8, H], BF16, kind="ExternalInput")
        Mem = nc.dram_tensor("Mem", [P, KC8, MEM], BF16, kind="ExternalInput")
    else:
        MQ = nc.dram_tensor("MQ", [P, KC4, 2, MEM + H], F8, kind="ExternalInput")
    WU = nc.dram_tensor("WU", [P, KC4, 2, 2 * H], F8, kind="ExternalInput")
    Mt = nc.dram_tensor("Mt", [P, KC8, NB], F32R, kind="ExternalInput")
    if not zero_cb:
        cbP = nc.dram_tensor("cbP", [P, H], F32, kind="ExternalInput")
    if not zero_affine:
        lngP = nc.dram_tensor("lngP", [P, H], F32, kind="ExternalInput")
        lnbP = nc.dram_tensor("lnbP", [P, H], F32, kind="ExternalInput")
    negmb = nc.dram_tensor("negmb", [NB], F32, kind="ExternalInput")
    powers = nc.dram_tensor("powers", [NB], F32, kind="ExternalInput")
    y = nc.dram_tensor("y", [BC, H], BF16, kind="ExternalOutput")
    R = nc.dram_tensor("Rtab", [MEM, H], BF16, kind="Internal")
    scr = nc.dram_tensor("scr", [BT * P], I32, kind="Internal")
    dbg = bool(int(_os.environ.get("K_DEBUG", "0")))
    if dbg:
        idx_out = nc.dram_tensor("idx_out", [BT * P], I32, kind="ExternalOutput")
        R_out = nc.dram_tensor("R_out", [MEM, H], BF16, kind="ExternalOutput")
    wsink_d = nc.dram_tensor("wsink", [P, 1], F32, kind="Internal")
    y_ap = y.ap()
    R_ap = R.ap()
    scr_ap = scr.ap()

    rscale = INV_WSCALE  # psum holds 64x the true product
    N_NEWT = int(_os.environ.get("K_NEWT", "1"))

    with tile.TileContext(nc) as tc, ExitStack() as ctx:
        wpool = ctx.enter_context(tc.tile_pool(name="weights", bufs=1))
        work = ctx.enter_context(tc.tile_pool(name="work", bufs=3))
        rgp = ctx.enter_context(tc.tile_pool(name="rgp", bufs=8))
        hpool = ctx.enter_context(tc.tile_pool(name="hpool", bufs=2))
        ipool = ctx.enter_context(tc.tile_pool(name="ipool", bufs=2))
        epil = ctx.enter_context(tc.tile_pool(name="epil", bufs=3))
        small = ctx.enter_context(tc.tile_pool(name="small", bufs=8))
        psum = ctx.enter_context(tc.tile_pool(name="psum", bufs=2, space="PSUM"))

        # ---- warmup: dummy matmuls with no DMA deps keep the PE busy and
        # trip the HAM clock gate while the first weight chunks stream in.
        wu_l = wpool.tile([P, 2, P], F8)
        wu_r = wpool.tile([P, 2, 512], F8)
        nc.gpsimd.memset(wu_l[:], 0)
        nc.gpsimd.memset(wu_r[:], 0)
        n_wu = int(_os.environ.get("K_WU", "8"))
        ps_w = psum.tile([P, 512], F32, tag="psL", space="PSUM")
        for _ in range(n_wu):
            nc.tensor.matmul(out=ps_w[:], lhsT=wu_l[:], rhs=wu_r[:],
                             start=True, stop=True, perf_mode=DR)
        wsink = wpool.tile([P, 1], F32)
        nc.vector.tensor_copy(out=wsink[:], in_=ps_w[:, 0:1])
        nc.scalar.dma_start(out=wsink_d.ap()[:, :], in_=wsink[:])

        # ---- resident constants; mem/q first so the R phase starts early
        if r_bf16:
            mem_sb = wpool.tile([P, KC8, MEM], BF16)
            q_sb = wpool.tile([P, KC8, H], BF16)
            for kc in range(KC8):
                nc.sync.dma_start(out=mem_sb[:, kc, :], in_=Mem.ap()[:, kc, :])
                nc.sync.dma_start(out=q_sb[:, kc, :], in_=Qt.ap()[:, kc, :])
        else:
            # single DMA: one completion sem gates every R-phase LDW and MM
            # together (a stalled DoubleRow matmul with runnable successor
            # LDWEIGHTS clobbers the PE weight buffer).
            mq_sb = wpool.tile([P, KC4, 2, MEM + H], F8)
            for kc in range(KC4):
                nc.sync.dma_start(out=mq_sb[:, kc, :, :],
                                  in_=MQ.ap()[:, kc, :, :])
            mem_sb = mq_sb
            q_sb = mq_sb
        wu_sb = wpool.tile([P, KC4, 2, 2 * H], F8)
        for kc in range(KC4):
            nc.sync.dma_start(out=wu_sb[:, kc, :, :], in_=WU.ap()[:, kc, :, :])
        w_sb = wu_sb
        u_sb = wu_sb
        mw_sb = wpool.tile([P, KC8, NB], F32R)
        nc.sync.dma_start(out=mw_sb[:], in_=Mt.ap()[:, :, :])

        if not zero_cb:
            cbb = wpool.tile([P, H], F32)
            nc.sync.dma_start(out=cbb[:], in_=cbP.ap())
        if not zero_affine:
            gb = wpool.tile([P, H], F32)
            bb = wpool.tile([P, H], F32)
            nc.sync.dma_start(out=gb[:], in_=lngP.ap())
            nc.sync.dma_start(out=bb[:], in_=lnbP.ap())
        eps = wpool.tile([P, 1], F32)
        nc.vector.memset(eps[:], LN_EPS)
        nmb_c = wpool.tile([NB, 1], F32)
        pw_c = wpool.tile([NB, 1], F32)
        nc.sync.dma_start(out=nmb_c[:], in_=negmb.ap()[:, None])
        nc.sync.dma_start(out=pw_c[:], in_=powers.ap()[:, None])
        ones10 = wpool.tile([NB, 1], BF16)
        nc.vector.memset(ones10[:], 1.0)

        # ---- phase 1: R = (memory @ Q_w.T) * 2^-6 + combined_bias ----
        # mt-outer, halves alternate psA/psB so the drain of one unit
        # overlaps the matmuls of the next.
        for mt in range(KC8):
            for half in range(2):
                hs = slice(half * 512, (half + 1) * 512)
                tag = "psA" if half == 0 else "psB"
                ps_r = psum.tile([P, 512], F32, tag=tag, space="PSUM", bufs=3)
                if r_bf16:
                    for kc in range(KC8):
                        nc.tensor.matmul(out=ps_r[:],
                                         lhsT=mem_sb[:, kc, mt * P:(mt + 1) * P],
                                         rhs=q_sb[:, kc, hs],
                                         start=(kc == 0), stop=(kc == KC8 - 1))
                else:
                    qs = slice(MEM + half * 512, MEM + (half + 1) * 512)
                    for kc in range(KC4):
                        nc.tensor.matmul(out=ps_r[:],
                                         lhsT=mem_sb[:, kc, :, mt * P:(mt + 1) * P],
                                         rhs=q_sb[:, kc, :, qs],
                                         start=(kc == 0), stop=(kc == KC4 - 1),
                                         perf_mode=DR)
                r_sb = work.tile([P, 512], BF16, tag="rtile")
                if zero_cb:
                    nc.vector.tensor_scalar(
                        out=r_sb[:], in0=ps_r[:], scalar1=rscale, scalar2=None,
                        op0=mybir.AluOpType.mult)
                else:
                    nc.vector.scalar_tensor_tensor(
                        out=r_sb[:], in0=ps_r[:], scalar=rscale, in1=cbb[:, hs],
                        op0=mybir.AluOpType.mult, op1=mybir.AluOpType.add)
                nc.scalar.dma_start(out=R_ap[mt * P:(mt + 1) * P, hs], in_=r_sb[:])
                if dbg:
                    nc.scalar.dma_start(out=R_out.ap()[mt * P:(mt + 1) * P, hs],
                                        in_=r_sb[:])

        # ---- phase 2: 4 groups of 4 b-tiles ----
        def tile_mms(bt):
            """DMA + the 16 DoubleRow matmuls for one b-tile; returns psums."""
            xb = work.tile([P, KC4, 2, P], F8, tag="xb")
            hb = work.tile([P, KC4, 2, P], F8, tag="hb")
            nc.sync.dma_start(out=xb[:], in_=xT.ap()[:, bt, :, :, :])
            nc.sync.dma_start(out=hb[:], in_=hT.ap()[:, bt, :, :, :])
            ps0 = psum.tile([P, 512], F32, tag="psA", space="PSUM", bufs=3)
            ps1 = psum.tile([P, 512], F32, tag="psB", space="PSUM", bufs=3)
            for kc in range(KC4):
                nc.tensor.matmul(out=ps0[:], lhsT=xb[:, kc, :, :],
                                 rhs=w_sb[:, kc, :, 0:512],
                                 start=(kc == 0), stop=False, perf_mode=DR)
                nc.tensor.matmul(out=ps1[:], lhsT=xb[:, kc, :, :],
                                 rhs=w_sb[:, kc, :, 512:1024],
                                 start=(kc == 0), stop=False, perf_mode=DR)
            for kc in range(KC4):
                nc.tensor.matmul(out=ps0[:], lhsT=hb[:, kc, :, :],
                                 rhs=u_sb[:, kc, :, H:H + 512],
                                 start=False, stop=(kc == KC4 - 1), perf_mode=DR)
                nc.tensor.matmul(out=ps1[:], lhsT=hb[:, kc, :, :],
                                 rhs=u_sb[:, kc, :, H + 512:2 * H],
                                 start=False, stop=(kc == KC4 - 1), perf_mode=DR)
            return ps0, ps1

        def tile_epilogue(bt, ps0, ps1, rg):
            pre = epil.tile([P, H], BF16, tag="pre")
            nc.vector.scalar_tensor_tensor(
                out=pre[:, 0:512], in0=ps0[:], scalar=rscale, in1=rg[:, 0:512],
                op0=mybir.AluOpType.mult, op1=mybir.AluOpType.add)
            nc.vector.scalar_tensor_tensor(
                out=pre[:, 512:1024], in0=ps1[:], scalar=rscale,
                in1=rg[:, 512:1024],
                op0=mybir.AluOpType.mult, op1=mybir.AluOpType.add)

            stats = small.tile([P, 2, 6], F32, tag="stats")
            mv = small.tile([P, 2], F32, tag="mv")
            nc.vector.bn_stats(out=stats[:, 0, :], in_=pre[:, 0:512])
            nc.vector.bn_stats(out=stats[:, 1, :], in_=pre[:, 512:1024])
            nc.vector.bn_aggr(out=mv[:], in_=stats[:])

            # rstd = 1/sqrt(var+eps): magic-constant + Newton on DVE.
            # (ACT Sqrt would thrash the activation table against Sigmoid.)
            v = small.tile([P, 1], F32, tag="v")
            ri = small.tile([P, 1], I32, tag="ri")
            t = small.tile([P, 1], F32, tag="t")
            ry = ri[:].bitcast(F32)
            nc.vector.tensor_scalar_add(out=v[:], in0=mv[:, 1:2],
                                        scalar1=LN_EPS)
            nc.vector.tensor_scalar(out=ri[:], in0=v[:].bitcast(I32),
                                    scalar1=1, scalar2=None,
                                    op0=mybir.AluOpType.arith_shift_right)
            nc.vector.tensor_scalar(out=ri[:], in0=ri[:], scalar1=0,
                                    scalar2=None,
                                    op0=mybir.AluOpType.bitwise_not)
            nc.vector.tensor_scalar(out=ri[:], in0=ri[:],
                                    scalar1=0x5F3759E0, scalar2=None,
                                    op0=mybir.AluOpType.add)
            for _ in range(N_NEWT):
                nc.vector.scalar_tensor_tensor(
                    out=t[:], in0=ry, scalar=ry, in1=v[:],
                    op0=mybir.AluOpType.mult, op1=mybir.AluOpType.mult)
                nc.vector.tensor_scalar(out=t[:], in0=t[:], scalar1=-0.5,
                                        scalar2=1.5,
                                        op0=mybir.AluOpType.mult,
                                        op1=mybir.AluOpType.add)
                nc.vector.tensor_tensor(out=ry, in0=ry, in1=t[:],
                                        op=mybir.AluOpType.mult)
            rstd = ri[:].bitcast(F32)
            if zero_affine:
                nmr = small.tile([P, 1], F32, tag="nmr")
                nc.vector.scalar_tensor_tensor(
                    out=nmr[:], in0=mv[:, 0:1], scalar=-1.0, in1=rstd,
                    op0=mybir.AluOpType.mult, op1=mybir.AluOpType.mult)
                ob = work.tile([P, H], BF16, tag="ob")
                nc.scalar.activation(out=ob[:], in_=pre[:],
                                     func=mybir.ActivationFunctionType.Sigmoid,
                                     bias=nmr[:], scale=rstd)
                nc.scalar.dma_start(out=y_ap[bt * P:(bt + 1) * P, :], in_=ob[:])
            else:
                t = epil.tile([P, H], F32, tag="tgen")
                nc.vector.scalar_tensor_tensor(
                    out=t[:], in0=pre[:], scalar=mv[:, 0:1], in1=gb[:],
                    op0=mybir.AluOpType.subtract, op1=mybir.AluOpType.mult)
                nc.vector.scalar_tensor_tensor(
                    out=t[:], in0=t[:], scalar=rstd, in1=bb[:],
                    op0=mybir.AluOpType.mult, op1=mybir.AluOpType.add)
                ob = work.tile([P, H], BF16, tag="ob")
                nc.scalar.activation(out=ob[:], in_=t[:],
                                     func=mybir.ActivationFunctionType.Sigmoid)
                nc.scalar.dma_start(out=y_ap[bt * P:(bt + 1) * P, :], in_=ob[:])

        def logits_a(g0):
            """hfg loads + transposed fp32r logits matmuls for 4 b-tiles."""
            n = GROUP * P
            hfg = hpool.tile([P, KC8, GROUP, P], F32R, tag="hfg")
            for kc in range(KC8):
                nc.sync.dma_start(out=hfg[:, kc, :, :],
                                  in_=hTf.ap()[:, g0 // GROUP * GROUP:
                                               g0 // GROUP * GROUP + GROUP,
                                               kc, :])
            psLT = psum.tile([NB, 512], F32, tag="psL", space="PSUM")
            for kc in range(KC8):
                nc.tensor.matmul(out=psLT[:, 0:n], lhsT=mw_sb[:, kc, :],
                                 rhs=hfg[:, kc, :, :], start=(kc == 0),
                                 stop=(kc == KC8 - 1))
            return psLT

        def logits_b(g0, psLT):
            """bits -> index -> partition-major indices -> gathers."""
            n = GROUP * P
            bitsT = ipool.tile([NB, 512], BF16, tag="bitsT")
            nc.vector.tensor_scalar(out=bitsT[:, 0:n], in0=psLT[:, 0:n],
                                    scalar1=nmb_c[:], scalar2=pw_c[:],
                                    op0=mybir.AluOpType.is_gt,
                                    op1=mybir.AluOpType.mult)
            psI = psum.tile([1, 512], F32, tag="psL", space="PSUM")
            nc.tensor.matmul(out=psI[0:1, 0:n], lhsT=ones10[:],
                             rhs=bitsT[:, 0:n], start=True, stop=True)
            idxf = ipool.tile([1, 512], I32, tag="idxf")
            nc.vector.tensor_copy(out=idxf[0:1, 0:n], in_=psI[0:1, 0:n])
            # DRAM bounce: [1, 512] free-axis -> [128, 4] partition-major
            nc.gpsimd.dma_start(out=scr_ap[None, g0 * P:(g0 + GROUP) * P],
                                in_=idxf[0:1, 0:n])
            if dbg:
                nc.scalar.dma_start(
                    out=idx_out.ap()[None, g0 * P:(g0 + GROUP) * P],
                    in_=idxf[0:1, 0:n])
            idxT = small.tile([P, GROUP], I32, tag="idxT")
            tr_ap = bass.AP(tensor=scr_ap.tensor, offset=g0 * P,
                            ap=[[1, P], [P, GROUP]])
            nc.gpsimd.dma_start(out=idxT[:, :], in_=tr_ap)
            rgs = []
            for tb in range(GROUP):
                rg = rgp.tile([P, H], BF16, tag="rg")
                nc.gpsimd.indirect_dma_start(
                    out=rg[:], out_offset=None, in_=R_ap[:, :],
                    in_offset=bass.IndirectOffsetOnAxis(
                        ap=idxT[:, tb:tb + 1], axis=0))
                rgs.append(rg)
            return rgs

        psLT0 = logits_a(0)
        rgs_cur = logits_b(0, psLT0)
        nxt_psLT = None
        rgs_next = None
        for g in range(BT // GROUP):
            for tb in range(GROUP):
                bt = g * GROUP + tb
                ps0, ps1 = tile_mms(bt)
                if tb == 1 and g + 1 < BT // GROUP:
                    nxt_psLT = logits_a((g + 1) * GROUP)
                if tb == 2 and g + 1 < BT // GROUP:
                    rgs_next = logits_b((g + 1) * GROUP, nxt_psLT)
                tile_epilogue(bt, ps0, ps1, rgs_cur[tb])
            rgs_cur = rgs_next

    nc.compile()
    return nc


import os as _os

FLAGS = {
    "r_bf16": bool(int(_os.environ.get("K_RBF16", "0"))),
}


def _get_nc(zero_affine=True, zero_cb=True):
    key = ("nc", zero_affine, zero_cb, tuple(sorted(FLAGS.items())))
    if key not in _CACHE:
        _CACHE[key] = build_nc(zero_affine, zero_cb=zero_cb, **FLAGS)
    return _CACHE[key]


def _pack_act8(a):
    """[BC, 1024] -> [p, bt, kc, ko, bp] fp8, k = kc*256 + ko*128 + p."""
    t = a.reshape(BT, P, KC4, 2, P).transpose(4, 0, 2, 3, 1)
    return np.ascontiguousarray(t).astype(F8NP)


def _pack_w8(w, scale=WSCALE):
    """[n, 1024] -> [p, kc, ko, n] fp8 scaled."""
    t = w.T.reshape(KC4, 2, P, -1).transpose(2, 0, 1, 3)
    return np.clip(np.ascontiguousarray(t) * np.float32(scale),
                   -240, 240).astype(F8NP)


def _pack_wb(w):
    """[n, 1024] -> [p, kc8, n] bf16 (for r_bf16 fallback)."""
    t = w.T.reshape(KC8, P, -1).transpose(1, 0, 2)
    return np.ascontiguousarray(t).astype(BF16_NP)


def _pack_f32(a):
    """[BC, 1024] -> [p, bt, kc8, bp] f32."""
    return np.ascontiguousarray(a.reshape(BT, P, KC8, P).transpose(3, 0, 2, 1))


def prepare_in_maps(inputs):
    x = np.asarray(inputs["x"], np.float32)
    h = np.asarray(inputs["h_prev"], np.float32)
    memory = np.asarray(inputs["memory"], np.float32)
    W_w = np.asarray(inputs["W_w"], np.float32)
    U_w = np.asarray(inputs["U_w"], np.float32)
    Q_w = np.asarray(inputs["Q_w"], np.float32)
    M_w = np.asarray(inputs["M_w"], np.float32)
    W_b = np.asarray(inputs["W_b"], np.float32)
    U_b = np.asarray(inputs["U_b"], np.float32)
    Q_b = np.asarray(inputs["Q_b"], np.float32)
    M_b = np.asarray(inputs["M_b"], np.float32)
    ln_g = np.asarray(inputs["ln_g"], np.float32)
    ln_b = np.asarray(inputs["ln_b"], np.float32)

    cb = (W_b + U_b + Q_b).astype(np.float32)
    zero_cb = bool(np.all(cb == 0.0))
    shared = {
        "WU": np.concatenate([_pack_w8(W_w), _pack_w8(U_w)], axis=3),
        # Mt: [p, kc8, j] fp32 (fp32r bits)
        "Mt": np.ascontiguousarray(
            M_w.T.reshape(KC8, P, NB).transpose(1, 0, 2)),
        "negmb": np.ascontiguousarray(-M_b),
        "powers": (2.0 ** np.arange(NB - 1, -1, -1)).astype(np.float32),
    }
    if FLAGS["r_bf16"]:
        shared["Qt"] = _pack_wb(Q_w)
        shared["Mem"] = _pack_wb(memory)
    else:
        shared["MQ"] = np.concatenate(
            [_pack_w8(memory, scale=1.0), _pack_w8(Q_w)], axis=3)
    if not zero_cb:
        shared["cbP"] = np.ascontiguousarray(np.broadcast_to(cb, (P, H)))
    zero_affine = bool(np.all(ln_g == 1.0) and np.all(ln_b == 0.0))
    if not zero_affine:
        shared["lngP"] = np.ascontiguousarray(np.broadcast_to(ln_g, (P, H)))
        shared["lnbP"] = np.ascontiguousarray(np.broadcast_to(ln_b, (P, H)))
    in_maps = []
    for i in range(NCORES):
        sl = slice(i * BC, (i + 1) * BC)
        m = dict(shared)
        m["xT"] = _pack_act8(x[sl])
        m["hT"] = _pack_act8(h[sl])
        m["hTf"] = _pack_f32(h[sl])
        in_maps.append(m)
    return in_maps


def run(inputs, trace=False, trace_cores=None):
    zero_affine = bool(
        np.all(np.asarray(inputs["ln_g"], np.float32) == 1.0)
        and np.all(np.asarray(inputs["ln_b"], np.float32) == 0.0))
    zero_cb = bool(
        np.all(np.asarray(inputs["W_b"], np.float32) == 0.0)
        and np.all(np.asarray(inputs["U_b"], np.float32) == 0.0)
        and np.all(np.asarray(inputs["Q_b"], np.float32) == 0.0))
    nc = _get_nc(zero_affine, zero_cb)
    in_maps = prepare_in_maps(inputs)
    res = bass_utils.run_bass_kernel_spmd(
        nc, in_maps, core_ids=list(range(NCORES)), trace=trace,
        trace_cores=trace_cores)
    out = np.concatenate(
        [np.asarray(r["y"], dtype=np.float32) for r in res.results], axis=0)
    return out, res


def kernel(**inputs):
    out, _ = run(inputs)
    return out.astype(np.float32)


def enable_profiling():
    """Inject the missing antenv.axon_hooks shim so trace=True works, and
    neutralize the S3 artifact upload (zero-egress container)."""
    import sys
    import types
    try:
        import antenv.axon_hooks  # noqa: F401
    except ImportError:
        mod = types.ModuleType("antenv.axon_hooks")
        _hook = [None]
        mod.set_axon_ntff_profile_hook = lambda h: _hook.__setitem__(0, h)
        mod.get_axon_ntff_profile_hook = lambda: _hook[0]
        sys.modules["antenv.axon_hooks"] = mod
        from trn_agent_boot.trn_boot import _ntff_profile_via_ctypes
        mod.set_axon_ntff_profile_hook(
            _ntff_profile_via_ctypes("/opt/axon/libaxon_pjrt.so"))
    bass_utils.upload_artifacts = lambda d: "local://" + str(d)
